# revision 1
# baseline (speedup 1.0000x reference)
"""Trainium2 Bass kernel: BoundaryActivation.

Per sample: x1 = cummax(x, H), x2 = reverse-cummax(x, H), x3 = cummax(x, W),
x4 = reverse-cummax(x, W); out = conv1x1(concat([x, x1, x2, x3, x4])) + bias.

Sharding: data-parallel over batch, B=8 -> 8 NeuronCores, one sample each.

Per-core algorithm (sample x_s [256, 128, 128], flattened to [256, 16384]):
  - channel-in-partition layout [c_chunk(128), (h, w)]; matmul contracts
    channels (fp32r, full PE rate), N tiles of 512 spatial positions.
  - W-direction scans (x3/x4): tensor_tensor_scan along the free axis with a
    -inf "reset" bias every 128 elements (row starts); reverse via negative
    stride APs.
  - H-direction scans (x1/x2): strided-AP copy gathers a transposed band
    xT [c, (w8, h64)], scan along free with resets every 64, matmul in
    transposed spatial order into a separate PSUM group; the PSUM->SBUF copy
    un-transposes via a strided destination AP (free).
  - H is processed in two 64-row phases so SBUF holds only half maps. Suffix
    carries for the top half come from a column-max pre-pass over the bottom
    half; prefix carries for the bottom half come from the top half's last
    scan row. Carries are applied as an elementwise max AFTER the local scan
    (prefix-max with seed == max(unseeded prefix-max, seed)).
"""

import numpy as np
from contextlib import ExitStack

import concourse.bass as bass
import concourse.bacc as bacc
import concourse.mybir as mybir
import concourse.tile as tile
from concourse.bass_utils import run_bass_kernel_spmd

F32 = mybir.dt.float32
F32R = mybir.dt.float32r
AL = mybir.AluOpType
AFT = mybir.ActivationFunctionType

NEG = -3.0e38  # effective -inf for scan resets / initials

B = 8
C = 256
H = 128
W = 128
O = 256
NCC = 2          # channel chunks of 128
NQ = 2           # output-channel chunks of 128
HALF = 64        # rows per phase
BAND = 512       # matmul N-tile (spatial positions)
NBAND = 16       # hw bands per phase  (4 rows x 128 w each)
NTB = 16         # T bands per phase   (8 cols x 64 h each)
PRE_CHUNK = 1024  # pre-pass rows chunk (8 rows x 128 w)

# map index: 0=x, 1=x1(cummax H), 2=x2(revcummax H), 3=x3(cummax W), 4=x4(revcummax W)


def _w_col(m, cc, q):
    return ((m * NCC + cc) * NQ + q) * 128


def build_program():
    nc = bacc.Bacc()
    x_d = nc.declare_dram_parameter("x", [C, H * W], F32, isOutput=False)
    w_d = nc.declare_dram_parameter("wT", [5 * C, O], F32, isOutput=False)
    b_d = nc.declare_dram_parameter("bias", [O, 1], F32, isOutput=False)
    mA_d = nc.declare_dram_parameter("maskA", [128, BAND], F32, isOutput=False)
    mB_d = nc.declare_dram_parameter("maskB", [128, BAND], F32, isOutput=False)
    out_d = nc.declare_dram_parameter("out", [O, H * W], F32, isOutput=True)

    with ExitStack() as ctx:
        tc = ctx.enter_context(tile.TileContext(nc))

        const = ctx.enter_context(tc.tile_pool(name="const", bufs=1))
        persist = ctx.enter_context(tc.tile_pool(name="persist", bufs=1))
        xhalf_p = ctx.enter_context(tc.tile_pool(name="xhalf", bufs=2))
        tsurf_p = ctx.enter_context(tc.tile_pool(name="tsurf", bufs=1))
        stream_p = ctx.enter_context(tc.tile_pool(name="stream", bufs=2))
        pp_p = ctx.enter_context(tc.tile_pool(name="pp", bufs=4))
        xT_p = ctx.enter_context(tc.tile_pool(name="xT", bufs=2))
        x1T_p = ctx.enter_context(tc.tile_pool(name="x1T", bufs=3))
        x2T_p = ctx.enter_context(tc.tile_pool(name="x2T", bufs=3))
        x3_p = ctx.enter_context(tc.tile_pool(name="x3", bufs=3))
        x4_p = ctx.enter_context(tc.tile_pool(name="x4", bufs=3))
        outsb_p = ctx.enter_context(tc.tile_pool(name="outsb", bufs=4))
        psum_hw = ctx.enter_context(tc.tile_pool(name="psum_hw", bufs=4, space="PSUM"))
        psum_t = ctx.enter_context(tc.tile_pool(name="psum_t", bufs=4, space="PSUM"))

        # ---- constants ----
        wstage = const.tile([128, 20 * 128], F32, tag="wstage")
        # one DMA: wT[(tk p) o] -> [p, tk, o]; w_sb col layout tk*256 + q*128
        nc.sync.dma_start(
            wstage[:].rearrange("p (tk o) -> p tk o", o=O),
            w_d[:].rearrange("(tk p) o -> p tk o", p=128))
        w_sb = const.tile([128, 20 * 128], F32R, tag="w_sb")
        nc.scalar.activation(w_sb[:], wstage[:], AFT.Copy)
        maskA = const.tile([128, BAND], F32, tag="maskA")
        nc.sync.dma_start(maskA[:], mA_d[:])
        maskB = const.tile([128, BAND], F32, tag="maskB")
        nc.sync.dma_start(maskB[:], mB_d[:])
        bias_sb = const.tile([128, NQ], F32, tag="bias_sb")
        for q in range(NQ):
            nc.sync.dma_start(bias_sb[:, q:q + 1], b_d[q * 128:(q + 1) * 128, :])

        # carry tiles: column maxes per (chunk)  [128c, 128w]
        cmA = [persist.tile([128, W], F32R, tag=f"cmA{cc}", name=f"cmA{cc}") for cc in range(NCC)]
        cmB = [persist.tile([128, W], F32R, tag=f"cmB{cc}", name=f"cmB{cc}") for cc in range(NCC)]

        def w_ap(m, cc, q):
            return w_sb[:, _w_col(m, cc, q):_w_col(m, cc, q) + 128]

        # ---- pre-pass: column max of bottom half -> cmB ----
        for cc in range(NCC):
            nchunks = HALF * W // PRE_CHUNK  # 8
            acc = None
            for j in range(nchunks):
                t = stream_p.tile([128, PRE_CHUNK], F32, tag="stream", name="stream")
                src = x_d[cc * 128:(cc + 1) * 128,
                          HALF * W + j * PRE_CHUNK:HALF * W + (j + 1) * PRE_CHUNK]
                nc.gpsimd.dma_start(t[:], src)
                part = pp_p.tile([128, W], F32R, tag="pp", name="pp")
                # view (w outer, h inner); X-reduce over h
                v = t[:].rearrange("p (h w) -> p w h", w=W)
                nc.vector.tensor_reduce(part[:], v, mybir.AxisListType.X, AL.max)
                if acc is None:
                    acc = part
                else:
                    nc.vector.tensor_max(part[:], part[:], acc[:])
                    acc = part
            nc.vector.tensor_copy(cmB[cc][:], acc[:])

        # ---- phases ----
        for phase in ("A", "B"):
            h_off = 0 if phase == "A" else HALF
            col0 = h_off * W  # dram column offset of this phase

            xh = []
            for cc in range(NCC):
                t = xhalf_p.tile([128, HALF * W], F32, tag="xh", name="xh")
                nc.gpsimd.dma_start(
                    t[:], x_d[cc * 128:(cc + 1) * 128, col0:col0 + HALF * W])
                xh.append(t)

            tsurf = [tsurf_p.tile([128, HALF * W], F32, tag=f"ts{q}", name=f"ts{q}")
                     for q in range(NQ)]

            # ---- T path: x1 (cummax H), x2 (reverse cummax H) ----
            for tb in range(NTB):
                w0 = tb * 8
                x1T = {}
                x2T = {}
                xTd = {}
                for cc in range(NCC):
                    xT = xT_p.tile([128, BAND], F32R, tag="xT")
                    # gather transposed band: free = (w 8, h 64)
                    src = xh[cc][:].rearrange("p (h w) -> p w h", w=W)[:, w0:w0 + 8, :]
                    nc.scalar.activation(
                        xT[:].rearrange("p (w h) -> p w h", h=HALF), src, AFT.Copy)

                    t1 = x1T_p.tile([128, BAND], F32R, tag="x1T")
                    nc.vector.tensor_tensor_scan(
                        t1[:], maskB[:], xT[:], NEG, AL.add, AL.max)
                    t2 = x2T_p.tile([128, BAND], F32R, tag="x2T")
                    nc.vector.tensor_tensor_scan(
                        t2[:, ::-1], maskB[:], xT[:, ::-1], NEG, AL.add, AL.max)

                    if phase == "A":
                        # seed suffix-max with bottom-half column max
                        nc.vector.tensor_max(
                            t2[:].rearrange("p (w h) -> p w h", h=HALF),
                            t2[:].rearrange("p (w h) -> p w h", h=HALF),
                            cmB[cc][:, w0:w0 + 8].broadcast_to((128, 8, HALF)))
                        # harvest top-half column max for phase B prefix seed
                        nc.scalar.activation(
                            cmA[cc][:, w0:w0 + 8],
                            t1[:, HALF - 1::HALF], AFT.Copy)
                    else:
                        # seed prefix-max with top-half column max
                        nc.vector.tensor_max(
                            t1[:].rearrange("p (w h) -> p w h", h=HALF),
                            t1[:].rearrange("p (w h) -> p w h", h=HALF),
                            cmA[cc][:, w0:w0 + 8].broadcast_to((128, 8, HALF)))
                    x1T[cc] = t1
                    x2T[cc] = t2
                    xTd[cc] = xT

                for q in range(NQ):
                    pt = psum_t.tile([128, BAND], F32, tag="pt")
                    terms = [(0, 0, xTd[0]), (0, 1, xTd[1]),
                             (1, 0, x1T[0]), (1, 1, x1T[1]),
                             (2, 0, x2T[0]), (2, 1, x2T[1])]
                    for i, (m, cc, rhs) in enumerate(terms):
                        nc.tensor.matmul(
                            pt[:], w_ap(m, cc, q), rhs[:],
                            start=(i == 0), stop=(i == len(terms) - 1))
                    # un-transpose while copying PSUM -> SBUF surface
                    dst = tsurf[q][:].rearrange("p (h w) -> p w h", w=W)[:, w0:w0 + 8, :]
                    nc.scalar.activation(
                        dst, pt[:].rearrange("p (w h) -> p w h", h=HALF), AFT.Copy)

            # ---- hw path: x, x3 (cummax W), x4 (reverse cummax W) ----
            for b in range(NBAND):
                c0 = b * BAND
                x3 = {}
                x4 = {}
                for cc in range(NCC):
                    t3 = x3_p.tile([128, BAND], F32R, tag="x3")
                    nc.vector.tensor_tensor_scan(
                        t3[:], maskA[:], xh[cc][:, c0:c0 + BAND],
                        NEG, AL.add, AL.max)
                    t4 = x4_p.tile([128, BAND], F32R, tag="x4")
                    nc.vector.tensor_tensor_scan(
                        t4[:, ::-1], maskA[:], xh[cc][:, c0:c0 + BAND][:, ::-1],
                        NEG, AL.add, AL.max)
                    x3[cc] = t3
                    x4[cc] = t4

                for q in range(NQ):
                    ph = psum_hw.tile([128, BAND], F32, tag="ph")
                    terms = [(3, 0, x3[0][:]), (3, 1, x3[1][:]),
                             (4, 0, x4[0][:]), (4, 1, x4[1][:])]
                    for i, (m, cc, rhs) in enumerate(terms):
                        nc.tensor.matmul(
                            ph[:], w_ap(m, cc, q), rhs,
                            start=(i == 0), stop=(i == len(terms) - 1))
                    osb = outsb_p.tile([128, BAND], F32, tag="osb")
                    # out = (psum_hw + bias) + tsurf
                    nc.vector.scalar_tensor_tensor(
                        osb[:], ph[:], bias_sb[:, q:q + 1],
                        tsurf[q][:, c0:c0 + BAND], AL.add, AL.add)
                    nc.gpsimd.dma_start(
                        out_d[q * 128:(q + 1) * 128, col0 + c0:col0 + c0 + BAND],
                        osb[:])

    nc.finalize()
    return nc


_PROGRAM = None


def _get_program():
    global _PROGRAM
    if _PROGRAM is None:
        _PROGRAM = build_program()
    return _PROGRAM


def make_masks():
    mA = np.zeros((128, BAND), dtype=np.float32)
    mA[:, 0::128] = NEG
    mB = np.zeros((128, BAND), dtype=np.float32)
    mB[:, 0::64] = NEG
    return mA, mB


def make_in_maps(x, conv_w, conv_b):
    wT = np.ascontiguousarray(conv_w.T).astype(np.float32)      # [1280, 256]
    bias = conv_b.reshape(O, 1).astype(np.float32)
    mA, mB = make_masks()
    in_maps = []
    for i in range(B):
        in_maps.append({
            "x": np.ascontiguousarray(x[i].reshape(C, H * W)).astype(np.float32),
            "wT": wT,
            "bias": bias,
            "maskA": mA,
            "maskB": mB,
        })
    return in_maps


def kernel(x, conv_w, conv_b):
    nc = _get_program()
    in_maps = make_in_maps(x, conv_w, conv_b)
    res = run_bass_kernel_spmd(nc, in_maps, core_ids=list(range(B)))
    outs = [res.results[i]["out"].reshape(O, H, W) for i in range(B)]
    return np.stack(outs, axis=0).astype(np.float32)



# revision 2
# speedup vs baseline: 1.0465x; 1.0465x over previous
"""Trainium2 Bass kernel: BoundaryActivation, v10.

Per sample: x1 = cummax(x, H), x2 = reverse-cummax(x, H), x3 = cummax(x, W),
x4 = reverse-cummax(x, W); out = conv1x1(concat([x, x1, x2, x3, x4])) + bias.
Data-parallel over batch: B=8 -> 8 NeuronCores.

Per-core design (x_s [256, 16384] f32, out [256, 16384] f32 in T-ORDER
(col = w*128 + h); the host permutes back to row-major):

- Stream x in 16-row chunks [128, 2048] f32 per channel-chunk (cc).
  Per chunk: ACT gathers a transposed bf16 copy into the resident xT
  surface (col = w*128+h); DVE scans x3/x4 (cummax/rev-cummax along W,
  row-major, resets every 128, f32-in -> bf16-out); PE accumulates the
  "hw group" (x + x3 + x4 terms, 6 matmuls, bf16) per 4-row tile into
  PSUM; ACT copies PSUM -> osb_hw bf16 surface with the conv bias folded
  in (Identity activation + per-partition bias AP).
- Post-stream "T phase": DVE scans x1/x2 over xT per w16-band (runs of
  128, no seeds/carries -- full H is in one run); per w4-tile PE
  accumulates x1/x2 terms (packed rhs) plus an identity matmul whose rhs
  is a strided view of osb_hw (adds the hw partial + bias into the same
  PSUM); ACT copies PSUM -> outstage f32 (T-order, packed); DMA out.

All matmul moving operands and weights are bf16 (NeuronCC rejects mixed
32/16-bit matmuls); PSUM accumulates fp32. Max-scans are exact given
bf16 inputs, so the only error is bf16 input/weight rounding (~0.3%).
"""

import numpy as np
from contextlib import ExitStack

import concourse.bass as bass
import concourse.bacc as bacc
import concourse.mybir as mybir
import concourse.tile as tile
from concourse.bass_utils import run_bass_kernel_spmd

F32 = mybir.dt.float32
BF16 = mybir.dt.bfloat16
AL = mybir.AluOpType
AFT = mybir.ActivationFunctionType

NEG = -3.0e38

B = 8
C = 256
H = 128
W = 128
O = 256
HW = H * W            # 16384
NCC = 2               # input channel chunks of 128
NQ = 2                # output channel chunks of 128
CH_ROWS = 8           # rows per streamed chunk
NCHUNK = H // CH_ROWS  # 8
CHW = CH_ROWS * W     # 2048 cols per chunk
TROWS = 4             # hw-tile rows (psum tile = 4 rows x 128 w = 512)
NT = CH_ROWS // TROWS  # hw tiles per chunk (4)
WB = 16               # w-band width for T scans (free 2048)
NWB = W // WB         # 8
WT = 4                # T-tile width (psum tile = 4 w x 128 h = 512)
NWT = WB // WT        # T tiles per band (4)


def _wcol(m, cc, q):
    # w_sb column layout: contraction chunk tk = m*NCC+cc, then q
    return ((m * NCC + cc) * NQ + q) * 128


def build_program():
    nc = bacc.Bacc()
    x_d = nc.declare_dram_parameter("x", [C, HW], F32, isOutput=False)
    w_d = nc.declare_dram_parameter("wT", [5 * C, O], F32, isOutput=False)
    b_d = nc.declare_dram_parameter("bias", [O, 1], F32, isOutput=False)
    eye_d = nc.declare_dram_parameter("eye", [128, 128], F32, isOutput=False)
    out_d = nc.declare_dram_parameter("out", [O, HW], F32, isOutput=True)

    with ExitStack() as ctx:
        tc = ctx.enter_context(tile.TileContext(nc))

        const = ctx.enter_context(tc.tile_pool(name="const", bufs=1))
        persist = ctx.enter_context(tc.tile_pool(name="persist", bufs=1))
        stage_p = ctx.enter_context(tc.tile_pool(name="stage", bufs=3))
        x3_p = ctx.enter_context(tc.tile_pool(name="x3", bufs=4))
        x4_p = ctx.enter_context(tc.tile_pool(name="x4", bufs=4))
        x1_p = ctx.enter_context(tc.tile_pool(name="x1", bufs=3))
        x2_p = ctx.enter_context(tc.tile_pool(name="x2", bufs=3))
        outs_p = ctx.enter_context(tc.tile_pool(name="outs", bufs=2))
        psum_hw = ctx.enter_context(tc.tile_pool(name="psum_hw", bufs=4, space="PSUM"))
        psum_t = ctx.enter_context(tc.tile_pool(name="psum_t", bufs=4, space="PSUM"))

        # ---- constants ----
        # weights: wT [1280, 256] f32 -> w_sb bf16 [128, 20*128], staged in
        # two halves through the stream pool; col layout tk*256 + q*128
        w_sb = const.tile([128, 10 * 256], BF16, tag="w_sb")
        for half in range(2):
            ws = stage_p.tile([128, 5 * 256], F32, tag="stage", name="wstage")
            nc.sync.dma_start(
                ws[:].rearrange("p (tk o) -> p tk o", o=O),
                w_d[half * 640:(half + 1) * 640, :].rearrange(
                    "(tk p) o -> p tk o", p=128))
            nc.scalar.activation(
                w_sb[:, half * 1280:(half + 1) * 1280], ws[:], AFT.Copy)

        eyef = const.tile([128, 128], F32, tag="eyef")
        nc.sync.dma_start(eyef[:], eye_d[:])
        eye = const.tile([128, 128], BF16, tag="eye")
        nc.scalar.activation(eye[:], eyef[:], AFT.Copy)

        bias_sb = const.tile([128, NQ], F32, tag="bias_sb")
        for q in range(NQ):
            nc.sync.dma_start(bias_sb[:, q:q + 1], b_d[q * 128:(q + 1) * 128, :])

        # shared scan-reset mask: NEG at col % 128 == 0
        mask = const.tile([128, 2048], BF16, tag="mask")
        nc.vector.memset(mask[:], 0.0)
        nc.vector.memset(mask[:, 0::128], NEG)

        def w_ap(m, cc, q):
            c0 = _wcol(m, cc, q)
            return w_sb[:, c0:c0 + 128]

        # ---- residents ----
        # xT: T-order bf16 copy of x (col = w*128 + h), per cc
        xT = [persist.tile([128, HW], BF16, tag=f"xT{cc}", name=f"xT{cc}")
              for cc in range(NCC)]
        # osb_hw: hw-group partial (x + x3 + x4 terms + bias), row-major bf16
        osb = [persist.tile([128, HW], BF16, tag=f"osb{q}", name=f"osb{q}")
               for q in range(NQ)]

        # ---- stream phase ----
        for j in range(NCHUNK):
            h0 = j * CH_ROWS
            xs = {}
            x3t = {}
            x4t = {}
            for cc in range(NCC):
                st = stage_p.tile([128, CHW], F32, tag="stage", name="stage")
                nc.gpsimd.dma_start(
                    st[:], x_d[cc * 128:(cc + 1) * 128, h0 * W:(h0 + CH_ROWS) * W])
                xs[cc] = st
                # gather chunk into xT (convert f32->bf16)
                nc.scalar.activation(
                    xT[cc][:].rearrange("p (w h) -> p w h", h=H)[:, :, h0:h0 + CH_ROWS],
                    st[:].rearrange("p (h w) -> p w h", w=W),
                    AFT.Copy)
                # x3/x4: W-direction scans, row-major, f32 in -> bf16 out
                t3 = x3_p.tile([128, CHW], BF16, tag="x3", name="x3")
                nc.vector.tensor_tensor_scan(
                    t3[:], mask[:, :CHW], st[:], NEG, AL.add, AL.max)
                x3t[cc] = t3
                t4 = x4_p.tile([128, CHW], BF16, tag="x4", name="x4")
                nc.vector.tensor_tensor_scan(
                    t4[:, ::-1], mask[:, :CHW], st[:, ::-1], NEG, AL.add, AL.max)
                x4t[cc] = t4

            for t in range(NT):
                r0 = t * TROWS          # local row in chunk
                hh = h0 + r0            # global row
                for q in range(NQ):
                    pt = psum_hw.tile([128, TROWS * W], F32, tag="ph")
                    # x term: strided rhs from xT (cols w*128+h)
                    for i, cc in enumerate(range(NCC)):
                        nc.tensor.matmul(
                            pt[:].rearrange("p (h w) -> p h w", w=W),
                            w_ap(0, cc, q),
                            xT[cc][:].rearrange("p (w h) -> p h w", h=H)[:, hh:hh + TROWS, :],
                            start=(i == 0), stop=False)
                    # x3 / x4 terms: packed rhs from row-major scan tiles
                    terms = [(3, 0, x3t[0]), (3, 1, x3t[1]),
                             (4, 0, x4t[0]), (4, 1, x4t[1])]
                    for i, (m, cc, tt) in enumerate(terms):
                        nc.tensor.matmul(
                            pt[:], w_ap(m, cc, q),
                            tt[:, r0 * W:(r0 + TROWS) * W],
                            start=False, stop=(i == len(terms) - 1))
                    # PSUM -> osb_hw bf16 with bias folded in
                    nc.scalar.activation(
                        osb[q][:, hh * W:(hh + TROWS) * W], pt[:],
                        AFT.Identity, bias=bias_sb[:, q:q + 1])

        # ---- T phase ----
        for wb in range(NWB):
            w0 = wb * WB
            x1t = {}
            x2t = {}
            for cc in range(NCC):
                src = xT[cc][:, w0 * H:(w0 + WB) * H]
                t1 = x1_p.tile([128, WB * H], BF16, tag="x1", name="x1")
                nc.vector.tensor_tensor_scan(
                    t1[:], mask[:], src, NEG, AL.add, AL.max)
                x1t[cc] = t1
                t2 = x2_p.tile([128, WB * H], BF16, tag="x2", name="x2")
                nc.vector.tensor_tensor_scan(
                    t2[:, ::-1], mask[:], src[:, ::-1], NEG, AL.add, AL.max)
                x2t[cc] = t2

            for q in range(NQ):
                for half in range(2):
                    ot = outs_p.tile([128, WB * H // 2], F32, tag="outs", name="outs")
                    for wt in range(half * NWT // 2, (half + 1) * NWT // 2):
                        wl = wt * WT        # local w in band
                        wg = w0 + wl        # global w
                        ol = wl - half * (WB // 2)  # local w in outstage half
                        pt = psum_t.tile([128, WT * H], F32, tag="pt")
                        terms = [(1, 0, x1t[0]), (1, 1, x1t[1]),
                                 (2, 0, x2t[0]), (2, 1, x2t[1])]
                        for i, (m, cc, tt) in enumerate(terms):
                            nc.tensor.matmul(
                                pt[:], w_ap(m, cc, q),
                                tt[:, wl * H:(wl + WT) * H],
                                start=(i == 0), stop=False)
                        # identity matmul: add osb_hw (hw partial + bias), strided rhs
                        nc.tensor.matmul(
                            pt[:].rearrange("p (w h) -> p w h", h=H),
                            eye[:],
                            osb[q][:].rearrange("p (h w) -> p w h", w=W)[:, wg:wg + WT, :],
                            start=False, stop=True)
                        # PSUM -> outstage f32 (packed, T-order)
                        nc.scalar.activation(
                            ot[:, ol * H:(ol + WT) * H], pt[:], AFT.Copy)
                    nc.gpsimd.dma_start(
                        out_d[q * 128:(q + 1) * 128,
                              (w0 + half * WB // 2) * H:(w0 + (half + 1) * WB // 2) * H],
                        ot[:])

    nc.finalize()
    return nc


_PROGRAM = None


def _get_program():
    global _PROGRAM
    if _PROGRAM is None:
        _PROGRAM = build_program()
    return _PROGRAM


def make_in_maps(x, conv_w, conv_b):
    wT = np.ascontiguousarray(np.asarray(conv_w).T).astype(np.float32)  # [1280, 256]
    bias = np.asarray(conv_b).reshape(O, 1).astype(np.float32)
    eye = np.eye(128, dtype=np.float32)
    in_maps = []
    for i in range(B):
        in_maps.append({
            "x": np.ascontiguousarray(np.asarray(x[i]).reshape(C, HW)).astype(np.float32),
            "wT": wT,
            "bias": bias,
            "eye": eye,
        })
    return in_maps


def kernel(x, conv_w, conv_b):
    nc = _get_program()
    in_maps = make_in_maps(x, conv_w, conv_b)
    res = run_bass_kernel_spmd(nc, in_maps, core_ids=list(range(B)))
    outs = []
    for i in range(B):
        o = res.results[i]["out"].reshape(O, W, H)  # T-order: (o, w, h)
        outs.append(np.ascontiguousarray(o.transpose(0, 2, 1)))
    return np.stack(outs, axis=0).astype(np.float32)


# revision 3
# speedup vs baseline: 1.1543x; 1.1029x over previous
"""Trainium2 Bass kernel: BoundaryActivation, v10.

Per sample: x1 = cummax(x, H), x2 = reverse-cummax(x, H), x3 = cummax(x, W),
x4 = reverse-cummax(x, W); out = conv1x1(concat([x, x1, x2, x3, x4])) + bias.
Data-parallel over batch: B=8 -> 8 NeuronCores.

Per-core design (x_s [256, 16384] f32, out [256, 16384] f32 in T-ORDER
(col = w*128 + h); the host permutes back to row-major):

- Stream x in 16-row chunks [128, 2048] f32 per channel-chunk (cc).
  Per chunk: ACT gathers a transposed bf16 copy into the resident xT
  surface (col = w*128+h); DVE scans x3/x4 (cummax/rev-cummax along W,
  row-major, resets every 128, f32-in -> bf16-out); PE accumulates the
  "hw group" (x + x3 + x4 terms, 6 matmuls, bf16) per 4-row tile into
  PSUM; ACT copies PSUM -> osb_hw bf16 surface with the conv bias folded
  in (Identity activation + per-partition bias AP).
- Post-stream "T phase": DVE scans x1/x2 over xT per w16-band (runs of
  128, no seeds/carries -- full H is in one run); per w4-tile PE
  accumulates x1/x2 terms (packed rhs) plus an identity matmul whose rhs
  is a strided view of osb_hw (adds the hw partial + bias into the same
  PSUM); ACT copies PSUM -> outstage f32 (T-order, packed); DMA out.

All matmul moving operands and weights are bf16 (NeuronCC rejects mixed
32/16-bit matmuls); PSUM accumulates fp32. Max-scans are exact given
bf16 inputs, so the only error is bf16 input/weight rounding (~0.3%).
"""

import numpy as np
from contextlib import ExitStack

import concourse.bass as bass
import concourse.bacc as bacc
import concourse.mybir as mybir
import concourse.tile as tile
from concourse.bass_utils import run_bass_kernel_spmd

F32 = mybir.dt.float32
BF16 = mybir.dt.bfloat16
AL = mybir.AluOpType
AFT = mybir.ActivationFunctionType

NEG = -3.0e38

B = 8
C = 256
H = 128
W = 128
O = 256
HW = H * W            # 16384
NCC = 2               # input channel chunks of 128
NQ = 2                # output channel chunks of 128
CH_ROWS = 8           # rows per streamed chunk
NCHUNK = H // CH_ROWS  # 8
CHW = CH_ROWS * W     # 2048 cols per chunk
TROWS = 4             # hw-tile rows (psum tile = 4 rows x 128 w = 512)
NT = CH_ROWS // TROWS  # hw tiles per chunk (4)
WB = 16               # w-band width for T scans (free 2048)
NWB = W // WB         # 8
WT = 4                # T-tile width (psum tile = 4 w x 128 h = 512)
NWT = WB // WT        # T tiles per band (4)


def _wcol(m, cc, q):
    # w_sb column layout: contraction chunk tk = m*NCC+cc, then q
    return ((m * NCC + cc) * NQ + q) * 128


def build_program():
    nc = bacc.Bacc()
    x_d = nc.declare_dram_parameter("x", [C, HW], F32, isOutput=False)
    w_d = nc.declare_dram_parameter("wT", [5 * C, O], F32, isOutput=False)
    b_d = nc.declare_dram_parameter("bias", [O, 1], F32, isOutput=False)
    eye_d = nc.declare_dram_parameter("eye", [128, 128], F32, isOutput=False)
    out_d = nc.declare_dram_parameter("out", [O, HW], F32, isOutput=True)

    with ExitStack() as ctx:
        tc = ctx.enter_context(tile.TileContext(nc))

        const = ctx.enter_context(tc.tile_pool(name="const", bufs=1))
        persist = ctx.enter_context(tc.tile_pool(name="persist", bufs=1))
        stage_p = ctx.enter_context(tc.tile_pool(name="stage", bufs=4))
        x3_p = ctx.enter_context(tc.tile_pool(name="x3", bufs=3))
        x4_p = ctx.enter_context(tc.tile_pool(name="x4", bufs=3))
        x1_p = ctx.enter_context(tc.tile_pool(name="x1", bufs=3))
        x2_p = ctx.enter_context(tc.tile_pool(name="x2", bufs=3))
        outs_p = ctx.enter_context(tc.tile_pool(name="outs", bufs=5))
        psum_p = ctx.enter_context(tc.tile_pool(name="psum", bufs=8, space="PSUM"))

        # ---- constants ----
        # weights: wT [1280, 256] f32 -> w_sb bf16 [128, 20*128], staged in
        # two halves through the stream pool; col layout tk*256 + q*128
        w_sb = const.tile([128, 10 * 256], BF16, tag="w_sb")
        wstage_p = ctx.enter_context(tc.tile_pool(name="wstage", bufs=1))
        # stream-phase maps (x, x3, x4 = pieces 0, 3, 4) load first
        for piece in (0, 3, 4, 1, 2):
            ws = wstage_p.tile([128, 2 * 256], F32, tag="ws", name="ws")
            nc.sync.dma_start(
                ws[:].rearrange("p (tk o) -> p tk o", o=O),
                w_d[piece * 256:(piece + 1) * 256, :].rearrange(
                    "(tk p) o -> p tk o", p=128))
            nc.scalar.activation(
                w_sb[:, piece * 512:(piece + 1) * 512], ws[:], AFT.Copy)

        eyef = stage_p.tile([128, 128], F32, tag="stage", name="eyestage")
        nc.sync.dma_start(eyef[:], eye_d[:])
        eye = const.tile([128, 128], BF16, tag="eye")
        nc.scalar.activation(eye[:], eyef[:], AFT.Copy)

        bias_sb = const.tile([128, NQ], F32, tag="bias_sb")
        for q in range(NQ):
            nc.sync.dma_start(bias_sb[:, q:q + 1], b_d[q * 128:(q + 1) * 128, :])

        # shared scan-reset mask: NEG at col % 128 == 0
        mask = const.tile([128, 2048], BF16, tag="mask")
        nc.vector.memset(mask[:], 0.0)
        nc.vector.memset(mask[:, 0::128], NEG)

        def w_ap(m, cc, q):
            c0 = _wcol(m, cc, q)
            return w_sb[:, c0:c0 + 128]

        # ---- residents ----
        # xT: T-order bf16 copy of x (col = w*128 + h), per cc
        xT = [persist.tile([128, HW], BF16, tag=f"xT{cc}", name=f"xT{cc}")
              for cc in range(NCC)]
        # osb_hw: hw-group partial (x + x3 + x4 terms + bias), row-major bf16
        osb = [persist.tile([128, HW], BF16, tag=f"osb{q}", name=f"osb{q}")
               for q in range(NQ)]

        # ---- stream phase ----
        for j in range(NCHUNK):
            h0 = j * CH_ROWS
            xs = {}
            x3t = {}
            x4t = {}
            for cc in range(NCC):
                st = stage_p.tile([128, CHW], F32, tag="stage", name="stage")
                nc.gpsimd.dma_start(
                    st[:], x_d[cc * 128:(cc + 1) * 128, h0 * W:(h0 + CH_ROWS) * W])
                xs[cc] = st
                # gather chunk into xT (convert f32->bf16)
                nc.scalar.activation(
                    xT[cc][:].rearrange("p (w h) -> p w h", h=H)[:, :, h0:h0 + CH_ROWS],
                    st[:].rearrange("p (h w) -> p w h", w=W),
                    AFT.Copy)
                # x3/x4: W-direction scans, row-major, f32 in -> bf16 out
                t3 = x3_p.tile([128, CHW], BF16, tag="x3", name="x3")
                nc.vector.tensor_tensor_scan(
                    t3[:], mask[:, :CHW], st[:], NEG, AL.add, AL.max)
                x3t[cc] = t3
                t4 = x4_p.tile([128, CHW], BF16, tag="x4", name="x4")
                nc.vector.tensor_tensor_scan(
                    t4[:, ::-1], mask[:, :CHW], st[:, ::-1], NEG, AL.add, AL.max)
                x4t[cc] = t4

            for t in range(NT):
                r0 = t * TROWS          # local row in chunk
                hh = h0 + r0            # global row
                for q in range(NQ):
                    pt = psum_p.tile([128, TROWS * W], F32, tag="ps")
                    # x term: strided rhs from xT (cols w*128+h)
                    for i, cc in enumerate(range(NCC)):
                        nc.tensor.matmul(
                            pt[:].rearrange("p (h w) -> p h w", w=W),
                            w_ap(0, cc, q),
                            xT[cc][:].rearrange("p (w h) -> p h w", h=H)[:, hh:hh + TROWS, :],
                            start=(i == 0), stop=False)
                    # x3 / x4 terms: packed rhs from row-major scan tiles
                    terms = [(3, 0, x3t[0]), (3, 1, x3t[1]),
                             (4, 0, x4t[0]), (4, 1, x4t[1])]
                    for i, (m, cc, tt) in enumerate(terms):
                        nc.tensor.matmul(
                            pt[:], w_ap(m, cc, q),
                            tt[:, r0 * W:(r0 + TROWS) * W],
                            start=False, stop=(i == len(terms) - 1))
                    # PSUM -> osb_hw bf16 with bias folded in
                    nc.scalar.activation(
                        osb[q][:, hh * W:(hh + TROWS) * W], pt[:],
                        AFT.Identity, bias=bias_sb[:, q:q + 1])

        # ---- T phase ----
        for wb in range(NWB):
            w0 = wb * WB
            x1t = {}
            x2t = {}
            for cc in range(NCC):
                src = xT[cc][:, w0 * H:(w0 + WB) * H]
                t1 = x1_p.tile([128, WB * H], BF16, tag="x1", name="x1")
                nc.vector.tensor_tensor_scan(
                    t1[:], mask[:], src, NEG, AL.add, AL.max)
                x1t[cc] = t1
                t2 = x2_p.tile([128, WB * H], BF16, tag="x2", name="x2")
                nc.vector.tensor_tensor_scan(
                    t2[:, ::-1], mask[:], src[:, ::-1], NEG, AL.add, AL.max)
                x2t[cc] = t2

            for q in range(NQ):
                for wt in range(NWT):
                    wl = wt * WT        # local w in band
                    wg = w0 + wl        # global w
                    pt = psum_p.tile([128, WT * H], F32, tag="ps")
                    terms = [(1, 0, x1t[0]), (1, 1, x1t[1]),
                             (2, 0, x2t[0]), (2, 1, x2t[1])]
                    for i, (m, cc, tt) in enumerate(terms):
                        nc.tensor.matmul(
                            pt[:], w_ap(m, cc, q),
                            tt[:, wl * H:(wl + WT) * H],
                            start=(i == 0), stop=False)
                    # identity matmul: add osb_hw (hw partial + bias), strided rhs
                    nc.tensor.matmul(
                        pt[:].rearrange("p (w h) -> p w h", h=H),
                        eye[:],
                        osb[q][:].rearrange("p (h w) -> p w h", w=W)[:, wg:wg + WT, :],
                        start=False, stop=True)
                    # PSUM -> outstage f32 (packed, T-order)
                    ot = outs_p.tile([128, WT * H], F32, tag="outs", name="outs")
                    nc.scalar.activation(ot[:], pt[:], AFT.Copy)
                    eng = nc.gpsimd if q == 0 else nc.sync
                    eng.dma_start(
                        out_d[q * 128:(q + 1) * 128, wg * H:(wg + WT) * H],
                        ot[:])

    nc.finalize()
    return nc


_PROGRAM = None


def _get_program():
    global _PROGRAM
    if _PROGRAM is None:
        _PROGRAM = build_program()
    return _PROGRAM


def make_in_maps(x, conv_w, conv_b):
    wT = np.ascontiguousarray(np.asarray(conv_w).T).astype(np.float32)  # [1280, 256]
    bias = np.asarray(conv_b).reshape(O, 1).astype(np.float32)
    eye = np.eye(128, dtype=np.float32)
    in_maps = []
    for i in range(B):
        in_maps.append({
            "x": np.ascontiguousarray(np.asarray(x[i]).reshape(C, HW)).astype(np.float32),
            "wT": wT,
            "bias": bias,
            "eye": eye,
        })
    return in_maps


def kernel(x, conv_w, conv_b):
    nc = _get_program()
    in_maps = make_in_maps(x, conv_w, conv_b)
    res = run_bass_kernel_spmd(nc, in_maps, core_ids=list(range(B)))
    outs = []
    for i in range(B):
        o = res.results[i]["out"].reshape(O, W, H)  # T-order: (o, w, h)
        outs.append(np.ascontiguousarray(o.transpose(0, 2, 1)))
    return np.stack(outs, axis=0).astype(np.float32)
